# revision 37
# baseline (speedup 1.0000x reference)
"""Trainium2 Bass kernel for nn_Attention_71811853189409.

Module (per batch b of 16):
    xf   = x[b] reshaped [512, 4096]
    qkv  = w_qkv @ xf; q,k,v = split, viewed [8 heads, 64, 4096]
    q,k  l2-normalized along n=4096
    attn = softmax(scale * q_n @ k_n^T)            # [8, 64, 64]
    out  = attn @ v -> [512, 4096]
    y    = w_proj @ out + b_proj

Sharding: data-parallel over batch, 2 batches per core on 8 cores.

Per-core algorithm (big GEMMs with fp16 inputs / fp32 PSUM accum):
  P1: qkT [4096, 1024] = xf^T @ W_qk^T   (lhsT = xf tiles, natural layout;
      host interleaves W rows so qkT columns are [q0|k0|q1|k1|...])
  P2: per head h: Gram(Z_h), Z_h = qkT[:, 128h:128h+128] = [qT_h | kT_h]
      -> one [128,128] tile holding q@k^T AND diag blocks q@q^T, k@k^T
      (row norms come from the diagonals; no separate norm pass)
  P3: softmax on [64, 8, 64] tiles; 1/||q_i|| folded into the ACT Exp
      scale, row max into its bias, row sums via accum_out; 1/||k_j||
      broadcast along the free dim via a tiny DRAM bounce. attn written
      into blockdiag pair tiles; then the whole attention application
      and both projections collapse into one [512,512] matrix:
          M_pv = W_p @ blockdiag(attn) @ W_v
      built by 4 + 16 small matmuls entirely on-chip.
  P4: y = M_pv @ xf + b  (so v is never materialized; bias fused into
      the ACT evacuation; fp16 strips stored by ACT-ring DMAs, upcast
      to fp32 on the host).

Constraint discovered on this toolchain: every engine instruction may
carry AT MOST ONE semaphore wait. 16-bit matmuls split lhsT/rhs waits
across the LDWEIGHTS/MATMUL pair; all small tiles are per-batch
single-assignment; big tiles are double-buffered or have single-proc
fan-in; DMA rings are kept at <= 8 instructions (depth-1 lane model);
an SP nop chain at the end pre-observes all procs for the kernel drain.
"""

import numpy as np
from contextlib import ExitStack

import concourse.bass as bass
import concourse.mybir as mybir
import concourse.tile as tile
from concourse.bass_utils import run_bass_kernel_spmd

F32 = mybir.dt.float32
F16 = mybir.dt.float16
AF = mybir.ActivationFunctionType
MUL = mybir.AluOpType.mult

N_CORES = 8
B = 16
B_LOC = 1  # one batch per core per launch; two launches
C = 512
HW = 4096
HEADS = 8
D = 64
KT = 4          # k-tiles over C
NT = HW // 128  # 32 m-tiles over n
NB = HW // 512  # 8 n-banks of 512
SCALE = float(D) ** -0.5


def _build() -> bass.Bass:
    nc = bass.Bass(trn_type="TRN2")

    x = nc.dram_tensor("x", [B_LOC, C, HW], F16, kind="ExternalInput")
    # host-packed weight wall (see kernel()): [W_qk^T interleaved (1024)
    # | W_v natural (512) | W_p^T (512) | b_proj (1)] -> one load DMA
    WALL = 2 * C + C + C + 1
    wall = nc.dram_tensor("wall", [C, WALL], F16, kind="ExternalInput")
    ys = [nc.dram_tensor(f"y{b}", [C, HW], F16, kind="ExternalOutput")
          for b in range(B_LOC)]
    scr = [nc.dram_tensor(f"scr{b}", [D * HEADS], F32) for b in range(B_LOC)]

    tail: list = []

    with ExitStack() as ctx:
        tc = ctx.enter_context(tile.TileContext(nc))
        const = ctx.enter_context(tc.tile_pool(name="const", bufs=1))
        big = ctx.enter_context(tc.tile_pool(name="big", bufs=1))
        psA = ctx.enter_context(tc.tile_pool(name="psA", bufs=3, space="PSUM"))
        psD = ctx.enter_context(tc.tile_pool(name="psD", bufs=3, space="PSUM"))
        psg = ctx.enter_context(tc.tile_pool(name="psg", bufs=2, space="PSUM"))

        # ---- weights / constants (fp32 -> fp16 cast inside gpsimd DMA)
        wall_sb = const.tile([128, KT, WALL], F16)
        tail.append(nc.gpsimd.dma_start(
            out=wall_sb, in_=wall.rearrange("(k p) o -> p k o", p=128)))

        def wqk(k, sl):
            return wall_sb[:, k, sl]

        def wv_sl(k, sl):
            base = 2 * C
            return wall_sb[:, k, base + sl.start: base + sl.stop]

        def wp_sl(k, sl):
            base = 3 * C
            return wall_sb[:, k, base + sl.start: base + sl.stop]

        def bias_ap(ym):
            return wall_sb[:, ym, 4 * C:4 * C + 1]

        ident = const.tile([128, 128], F32)
        from concourse.masks import make_identity
        make_identity(nc, ident)

        # pre-touch DMA'd constants on their consuming engines
        bjunk = const.tile([128, 1], F16)
        nc.scalar.activation(bjunk, bias_ap(0), AF.Copy)    # ACT sees wall
        nc.tensor.ldweights(wall_sb[0:1, 0, 0:8])           # PE sees wall
        ijunk = const.tile([1, 8], F32)
        nc.vector.tensor_copy(ijunk, ident[0:1, 0:8])       # DVE sees ident

        # per-pair blockdiag attn tiles, zeroed once (off-diag stays 0)
        ap_tiles = []
        for hp in range(KT):
            t = const.tile([128, 128], F16, name=f"ap_{hp}")
            nc.gpsimd.memset(t, 0.0)
            nc.tensor.ldweights(t[0:1, 0:8])  # PE observes the memset once
            ap_tiles.append(t)

        mpT = const.tile([128, KT, C], F16)    # (W_p @ BD(attn))^T
        mpvT = const.tile([128, KT, C], F16)   # (W_p @ BD(attn) @ W_v)^T
        junk = const.tile([128, 128], F32)


        last_pe = last_act = last_dve = None

        for b in range(B_LOC):
            # ---- P1: load xf; qkT m-tiles feed PSUM-resident Grams -----
            xf = big.tile([128, KT, HW], F16, name="xf", tag="xf", bufs=2)
            tail.append(nc.gpsimd.dma_start(
                out=xf, in_=x[b].rearrange("(k p) n -> p k n", p=128)))

            # two PSUM tiles hold all 8 per-head Gram accumulators
            g0 = psg.tile([128, 512], F32, name="g0", tag="psg")
            g1 = psg.tile([128, 512], F32, name="g1", tag="psg")
            gtiles = [g0, g1]

            qkT = big.tile([128, NT, 2 * C], F16, name="qkT", tag="qkT")
            for m in range(NT):
                for h2 in range(2):
                    acc = psA.tile([128, 512], F32, name="acc_qk", tag="psA")
                    for k in range(KT):
                        last_pe = nc.tensor.matmul(
                            acc,
                            xf[:, k, m * 128:(m + 1) * 128],
                            wqk(k, slice(h2 * 512, (h2 + 1) * 512)),
                            start=(k == 0), stop=(k == KT - 1),
                        )
                    last_act = nc.scalar.activation(
                        qkT[:, m, h2 * 512:(h2 + 1) * 512], acc, AF.Copy)
                for h in range(HEADS):
                    z = qkT[:, m, h * 128:(h + 1) * 128]
                    # start=True only for the very first matmul of each
                    # bank (clears it); other heads' regions start fresh
                    # via per-element has_written bits
                    last_pe = nc.tensor.matmul(
                        gtiles[h // 4][:, (h % 4) * 128:(h % 4 + 1) * 128],
                        z, z,
                        start=(m == 0 and h % 4 == 0),
                        stop=(m == NT - 1),
                        skip_group_check=True,
                    )

            def gslice(h, rows=slice(0, 128), cols=slice(0, 128)):
                t = gtiles[h // 4]
                base = (h % 4) * 128
                return t[rows, base + cols.start: base + cols.stop]

            # ---- P3: softmax + M_pT + M_pvT (gram read from PSUM) ------
            # DVE pre-touch of the later-finishing gram tile absorbs the
            # PE wait so the diag-extract chain needs only DVE waits
            gt = const.tile([1, 8], F32, name=f"gt{b}")
            last_dve = nc.vector.tensor_copy(gt, g1[0:1, 0:8])
            d2 = const.tile([128, HEADS], F32, name=f"d2_{b}")
            for h in range(HEADS):
                last_dve = nc.vector.tensor_mul(junk, gslice(h), ident)
                last_dve = nc.vector.reduce_sum(
                    d2[:, h:h + 1], junk, axis=mybir.AxisListType.X)
            nrm = const.tile([128, HEADS], F32, name=f"nrm{b}")
            last_act = nc.scalar.activation(nrm, d2, AF.Sqrt)
            last_dve = nc.vector.tensor_scalar_max(nrm, nrm, 1e-12)
            rinv = const.tile([128, HEADS], F32, name=f"rinv{b}")
            last_dve = nc.vector.reciprocal(rinv, nrm)

            # bounce k-side 1/||k|| through DRAM to broadcast on free dim
            sc_ap = scr[b][:]
            st = nc.gpsimd.dma_start(
                out=sc_ap.rearrange("(h p) -> p h", p=D), in_=rinv[D:128, :])
            tail.append(st)
            rkrow = const.tile([D, HEADS, D], F32, name=f"rkrow{b}")
            bcast = bass.AP(
                tensor=sc_ap.tensor, offset=sc_ap.offset,
                ap=[[0, D], [1, HEADS * D]])
            rb = nc.gpsimd.dma_start(out=rkrow, in_=bcast)
            tail.append(rb)

            ss = const.tile([D, HEADS, D], F16, name=f"ss{b}")
            for half in range(2):
                gsrc = gtiles[half][0:D, :].rearrange(
                    "p (h c) -> p h c", h=4)[:, :, D:128]
                last_dve = nc.vector.tensor_tensor(
                    out=ss[:, half * 4:(half + 1) * 4, :], in0=gsrc,
                    in1=rkrow[:, half * 4:(half + 1) * 4, :], op=MUL)
            mx = const.tile([D, HEADS], F32, name=f"mx{b}")
            last_dve = nc.vector.reduce_max(mx, ss, axis=mybir.AxisListType.X)
            alpha = const.tile([D, HEADS], F32, name=f"alpha{b}")
            last_dve = nc.vector.tensor_scalar_mul(alpha, rinv[0:D, :], SCALE)
            beta = const.tile([D, HEADS], F32, name=f"beta{b}")
            last_dve = nc.vector.tensor_tensor(
                out=beta, in0=alpha, in1=mx, op=MUL)
            last_dve = nc.vector.tensor_scalar_mul(beta, beta, -1.0)

            ee = const.tile([D, HEADS, D], F16, name=f"ee{b}")
            esum = const.tile([D, HEADS], F32, name=f"esum{b}")
            for h in range(HEADS):
                last_act = nc.scalar.activation(
                    ee[:, h, :], ss[:, h, :], AF.Exp,
                    bias=beta[:, h:h + 1], scale=alpha[:, h:h + 1],
                    accum_out=esum[:, h:h + 1])
            rr = const.tile([D, HEADS], F32, name=f"rr{b}")
            last_dve = nc.vector.reciprocal(rr, esum)

            # M_pT[(h,e), c] = sum_d attn_h[d, e] * W_pT[(h,d), c]
            for hp in range(KT):  # 4 head pairs
                ap_t = ap_tiles[hp]
                last_dve = nc.vector.tensor_scalar_mul(
                    ap_t[0:D, 0:D], ee[:, 2 * hp, :], rr[:, 2 * hp:2 * hp + 1])
                last_dve = nc.vector.tensor_scalar_mul(
                    ap_t[D:128, D:128], ee[:, 2 * hp + 1, :],
                    rr[:, 2 * hp + 1:2 * hp + 2])
                acc = psD.tile([128, 512], F32, name="acc_mp", tag="psD")
                last_pe = nc.tensor.matmul(
                    acc, ap_t, wp_sl(hp, slice(0, C)), start=True, stop=True)
                last_dve = nc.vector.tensor_copy(mpT[:, hp, :], acc)

            # M_pvT[c', c] = sum_(he) W_v[(he), c'] * M_pT[(he), c]
            for cp in range(KT):
                acc = psD.tile([128, 512], F32, name="acc_mpv", tag="psD")
                for kt in range(KT):
                    last_pe = nc.tensor.matmul(
                        acc,
                        wv_sl(kt, slice(cp * 128, (cp + 1) * 128)),
                        mpT[:, kt, :],
                        start=(kt == 0), stop=(kt == KT - 1),
                    )
                last_dve = nc.vector.tensor_copy(mpvT[:, cp, :], acc)

            # ---- P4: y = M_pv @ xf + bias ------------------------------
            # single-use half-strip tiles: no reuse => no WAR/WAW waits;
            # stores alternate between the SP and ACT HWDGE rings so each
            # ring stays within its 8 lanes
            for ym in range(KT):
                for half in range(2):
                    yh = const.tile([128, HW // 2], F16,
                                    name=f"yh{b}_{ym}_{half}")
                    for nbi in range(NB // 2):
                        nb = half * (NB // 2) + nbi
                        acc = psA.tile([128, 512], F32, name="acc_y",
                                       tag="psA")
                        for kt in range(KT):
                            last_pe = nc.tensor.matmul(
                                acc,
                                mpvT[:, kt, ym * 128:(ym + 1) * 128],
                                xf[:, kt, nb * 512:(nb + 1) * 512],
                                start=(kt == 0), stop=(kt == KT - 1),
                            )
                        last_act = nc.scalar.activation(
                            yh[:, nbi * 512:(nbi + 1) * 512], acc,
                            AF.Identity, bias=bias_ap(ym))
                    eng = nc.sync if half == 0 else nc.scalar
                    tail.append(eng.dma_start(
                        out=ys[b][ym * 128:(ym + 1) * 128,
                                  half * (HW // 2):(half + 1) * (HW // 2)],
                        in_=yh))

        # ---- tail: SP observes every outstanding proc (1 wait per nop)
        for inst in [*tail, last_pe, last_act, last_dve]:
            if inst is None:
                continue
            n_ = nc.sync.nop(nofuse=True)
            tile.add_dep_helper(n_.ins, inst.ins, reason="tail observe")

    return nc


_NC_CACHE = None


def kernel(x, w_qkv, w_proj, b_proj):
    global _NC_CACHE
    if _NC_CACHE is None:
        _NC_CACHE = _build()
    nc = _NC_CACHE

    # one-pass fp32->fp16 cast (same rounding the on-device cast applied)
    x = np.asarray(x, dtype=np.float16).reshape(B, C, HW)
    w_qkv = np.asarray(w_qkv, dtype=np.float32)
    # interleave q_h / k_h row blocks so qkT columns are [q0|k0|q1|k1|...]
    perm = []
    for h in range(HEADS):
        perm.extend(range(h * D, (h + 1) * D))          # q_h rows
        perm.extend(range(C + h * D, C + (h + 1) * D))  # k_h rows
    w_qkT = w_qkv[perm].T                               # [512, 1024]
    w_v = w_qkv[2 * C:]                                 # [512, 512] natural
    w_pT = np.asarray(w_proj, dtype=np.float32).T
    b_col = np.asarray(b_proj, dtype=np.float32).reshape(C, 1)
    wall = np.ascontiguousarray(
        np.concatenate([w_qkT, w_v, w_pT, b_col], axis=1)).astype(
            np.float16)  # [512, 2049]; same rounding the on-device cast did

    outs = []
    for launch in range(2):
        in_maps = []
        for core in range(N_CORES):
            bi = launch * N_CORES + core
            in_maps.append({
                "x": np.ascontiguousarray(x[bi:bi + 1]),
                "wall": wall,
            })
        res = run_bass_kernel_spmd(nc, in_maps, core_ids=list(range(N_CORES)))
        outs.extend(r["y0"] for r in res.results)
    out = np.stack(outs)
    return out.reshape(B, C, 64, 64).astype(np.float32)


# revision 40
# speedup vs baseline: 1.0237x; 1.0237x over previous
"""Trainium2 Bass kernel for nn_Attention_71811853189409.

Module (per batch b of 16):
    xf   = x[b] reshaped [512, 4096]
    qkv  = w_qkv @ xf; q,k,v = split, viewed [8 heads, 64, 4096]
    q,k  l2-normalized along n=4096
    attn = softmax(scale * q_n @ k_n^T)            # [8, 64, 64]
    out  = attn @ v -> [512, 4096]
    y    = w_proj @ out + b_proj

Sharding: data-parallel over batch, 2 batches per core on 8 cores.

Per-core algorithm (big GEMMs with fp16 inputs / fp32 PSUM accum):
  P1: qkT [4096, 1024] = xf^T @ W_qk^T   (lhsT = xf tiles, natural layout;
      host interleaves W rows so qkT columns are [q0|k0|q1|k1|...])
  P2: per head h: Gram(Z_h), Z_h = qkT[:, 128h:128h+128] = [qT_h | kT_h]
      -> one [128,128] tile holding q@k^T AND diag blocks q@q^T, k@k^T
      (row norms come from the diagonals; no separate norm pass)
  P3: softmax on [64, 8, 64] tiles; 1/||q_i|| folded into the ACT Exp
      scale, row max into its bias, row sums via accum_out; 1/||k_j||
      broadcast along the free dim via a tiny DRAM bounce. attn written
      into blockdiag pair tiles; then the whole attention application
      and both projections collapse into one [512,512] matrix:
          M_pv = W_p @ blockdiag(attn) @ W_v
      built by 4 + 16 small matmuls entirely on-chip.
  P4: y = M_pv @ xf + b  (so v is never materialized; bias fused into
      the ACT evacuation; fp16 strips stored by ACT-ring DMAs, upcast
      to fp32 on the host).

Constraint discovered on this toolchain: every engine instruction may
carry AT MOST ONE semaphore wait. 16-bit matmuls split lhsT/rhs waits
across the LDWEIGHTS/MATMUL pair; all small tiles are per-batch
single-assignment; big tiles are double-buffered or have single-proc
fan-in; DMA rings are kept at <= 8 instructions (depth-1 lane model);
an SP nop chain at the end pre-observes all procs for the kernel drain.
"""

import numpy as np
from contextlib import ExitStack

import concourse.bass as bass
import concourse.mybir as mybir
import concourse.tile as tile
from concourse.bass_utils import run_bass_kernel_spmd

F32 = mybir.dt.float32
F16 = mybir.dt.float16
AF = mybir.ActivationFunctionType
MUL = mybir.AluOpType.mult

N_CORES = 8
B = 16
B_LOC = 1  # one batch per core per launch; two launches
C = 512
HW = 4096
HEADS = 8
D = 64
KT = 4          # k-tiles over C
NT = HW // 128  # 32 m-tiles over n
NB = HW // 512  # 8 n-banks of 512
SCALE = float(D) ** -0.5


def _build() -> bass.Bass:
    nc = bass.Bass(trn_type="TRN2")

    x = nc.dram_tensor("x", [B_LOC, C, HW], F16, kind="ExternalInput")
    # host-packed weight wall (see kernel()): [W_qk^T interleaved (1024)
    # | W_v natural (512) | W_p^T (512) | b_proj (1)] -> one load DMA
    WALL = 2 * C + C + C + 1
    wall = nc.dram_tensor("wall", [C, WALL], F16, kind="ExternalInput")
    ys = [nc.dram_tensor(f"y{b}", [C, HW], F16, kind="ExternalOutput")
          for b in range(B_LOC)]
    scr = [nc.dram_tensor(f"scr{b}", [D * HEADS], F32) for b in range(B_LOC)]

    tail: list = []

    with ExitStack() as ctx:
        tc = ctx.enter_context(tile.TileContext(nc))
        const = ctx.enter_context(tc.tile_pool(name="const", bufs=1))
        big = ctx.enter_context(tc.tile_pool(name="big", bufs=1))
        psA = ctx.enter_context(tc.tile_pool(name="psA", bufs=3, space="PSUM"))
        psD = ctx.enter_context(tc.tile_pool(name="psD", bufs=3, space="PSUM"))
        psg = ctx.enter_context(tc.tile_pool(name="psg", bufs=2, space="PSUM"))

        # ---- weights / constants (fp32 -> fp16 cast inside gpsimd DMA)
        wall_sb = const.tile([128, KT, WALL], F16)
        tail.append(nc.gpsimd.dma_start(
            out=wall_sb, in_=wall.rearrange("(k p) o -> p k o", p=128)))

        def wqk(k, sl):
            return wall_sb[:, k, sl]

        def wv_sl(k, sl):
            base = 2 * C
            return wall_sb[:, k, base + sl.start: base + sl.stop]

        def wp_sl(k, sl):
            base = 3 * C
            return wall_sb[:, k, base + sl.start: base + sl.stop]

        def bias_ap(ym):
            return wall_sb[:, ym, 4 * C:4 * C + 1]

        ident = const.tile([128, 128], F32)
        from concourse.masks import make_identity
        make_identity(nc, ident)

        # pre-touch DMA'd constants on their consuming engines
        bjunk = const.tile([128, 1], F16)
        nc.scalar.activation(bjunk, bias_ap(0), AF.Copy)    # ACT sees wall
        nc.tensor.ldweights(wall_sb[0:1, 0, 0:8])           # PE sees wall
        ijunk = const.tile([1, 8], F32)
        nc.vector.tensor_copy(ijunk, ident[0:1, 0:8])       # DVE sees ident

        # per-pair blockdiag attn tiles, zeroed once (off-diag stays 0)
        ap_tiles = []
        for hp in range(KT):
            t = const.tile([128, 128], F16, name=f"ap_{hp}")
            nc.gpsimd.memset(t, 0.0)
            nc.tensor.ldweights(t[0:1, 0:8])  # PE observes the memset once
            ap_tiles.append(t)

        mpT = const.tile([128, KT, C], F16)    # (W_p @ BD(attn))^T
        mpvT = const.tile([128, KT, C], F16)   # (W_p @ BD(attn) @ W_v)^T
        junk = const.tile([128, 128], F32)


        last_pe = last_act = last_dve = None

        for b in range(B_LOC):
            # ---- P1: load xf; qkT m-tiles feed PSUM-resident Grams -----
            xf = big.tile([128, KT, HW], F16, name="xf", tag="xf", bufs=2)
            tail.append(nc.sync.dma_start(
                out=xf, in_=x[b].rearrange("(k p) n -> p k n", p=128)))

            # two PSUM tiles hold all 8 per-head Gram accumulators
            g0 = psg.tile([128, 512], F32, name="g0", tag="psg")
            g1 = psg.tile([128, 512], F32, name="g1", tag="psg")
            gtiles = [g0, g1]

            qkT = big.tile([128, NT, 2 * C], F16, name="qkT", tag="qkT")
            for m in range(NT):
                for h2 in range(2):
                    acc = psA.tile([128, 512], F32, name="acc_qk", tag="psA")
                    for k in range(KT):
                        last_pe = nc.tensor.matmul(
                            acc,
                            xf[:, k, m * 128:(m + 1) * 128],
                            wqk(k, slice(h2 * 512, (h2 + 1) * 512)),
                            start=(k == 0), stop=(k == KT - 1),
                        )
                    last_act = nc.scalar.activation(
                        qkT[:, m, h2 * 512:(h2 + 1) * 512], acc, AF.Copy)
                for h in range(HEADS):
                    z = qkT[:, m, h * 128:(h + 1) * 128]
                    # start=True only for the very first matmul of each
                    # bank (clears it); other heads' regions start fresh
                    # via per-element has_written bits
                    last_pe = nc.tensor.matmul(
                        gtiles[h // 4][:, (h % 4) * 128:(h % 4 + 1) * 128],
                        z, z,
                        start=(m == 0 and h % 4 == 0),
                        stop=(m == NT - 1),
                        skip_group_check=True,
                    )

            def gslice(h, rows=slice(0, 128), cols=slice(0, 128)):
                t = gtiles[h // 4]
                base = (h % 4) * 128
                return t[rows, base + cols.start: base + cols.stop]

            # ---- P3: softmax + M_pT + M_pvT (gram read from PSUM) ------
            # DVE pre-touch of the later-finishing gram tile absorbs the
            # PE wait so the diag-extract chain needs only DVE waits
            gt = const.tile([1, 8], F32, name=f"gt{b}")
            last_dve = nc.vector.tensor_copy(gt, g1[0:1, 0:8])
            d2 = const.tile([128, HEADS], F32, name=f"d2_{b}")
            for h in range(HEADS):
                last_dve = nc.vector.tensor_mul(junk, gslice(h), ident)
                last_dve = nc.vector.reduce_sum(
                    d2[:, h:h + 1], junk, axis=mybir.AxisListType.X)
            nrm = const.tile([128, HEADS], F32, name=f"nrm{b}")
            last_act = nc.scalar.activation(nrm, d2, AF.Sqrt)
            last_dve = nc.vector.tensor_scalar_max(nrm, nrm, 1e-12)
            rinv = const.tile([128, HEADS], F32, name=f"rinv{b}")
            last_dve = nc.vector.reciprocal(rinv, nrm)

            # bounce k-side 1/||k|| through DRAM to broadcast on free dim
            sc_ap = scr[b][:]
            st = nc.gpsimd.dma_start(
                out=sc_ap.rearrange("(h p) -> p h", p=D), in_=rinv[D:128, :])
            tail.append(st)
            rkrow = const.tile([D, HEADS, D], F32, name=f"rkrow{b}")
            bcast = bass.AP(
                tensor=sc_ap.tensor, offset=sc_ap.offset,
                ap=[[0, D], [1, HEADS * D]])
            rb = nc.gpsimd.dma_start(out=rkrow, in_=bcast)
            tail.append(rb)

            ss = const.tile([D, HEADS, D], F16, name=f"ss{b}")
            for half in range(2):
                gsrc = gtiles[half][0:D, :].rearrange(
                    "p (h c) -> p h c", h=4)[:, :, D:128]
                last_dve = nc.vector.tensor_tensor(
                    out=ss[:, half * 4:(half + 1) * 4, :], in0=gsrc,
                    in1=rkrow[:, half * 4:(half + 1) * 4, :], op=MUL)
            mx = const.tile([D, HEADS], F32, name=f"mx{b}")
            last_dve = nc.vector.reduce_max(mx, ss, axis=mybir.AxisListType.X)
            alpha = const.tile([D, HEADS], F32, name=f"alpha{b}")
            last_dve = nc.vector.tensor_scalar_mul(alpha, rinv[0:D, :], SCALE)
            beta = const.tile([D, HEADS], F32, name=f"beta{b}")
            last_dve = nc.vector.tensor_tensor(
                out=beta, in0=alpha, in1=mx, op=MUL)
            last_dve = nc.vector.tensor_scalar_mul(beta, beta, -1.0)

            ee = const.tile([D, HEADS, D], F16, name=f"ee{b}")
            esum = const.tile([D, HEADS], F32, name=f"esum{b}")
            for h in range(HEADS):
                last_act = nc.scalar.activation(
                    ee[:, h, :], ss[:, h, :], AF.Exp,
                    bias=beta[:, h:h + 1], scale=alpha[:, h:h + 1],
                    accum_out=esum[:, h:h + 1])
            rr = const.tile([D, HEADS], F32, name=f"rr{b}")
            last_dve = nc.vector.reciprocal(rr, esum)

            # M_pT[(h,e), c] = sum_d attn_h[d, e] * W_pT[(h,d), c]
            for hp in range(KT):  # 4 head pairs
                ap_t = ap_tiles[hp]
                last_dve = nc.vector.tensor_scalar_mul(
                    ap_t[0:D, 0:D], ee[:, 2 * hp, :], rr[:, 2 * hp:2 * hp + 1])
                last_dve = nc.vector.tensor_scalar_mul(
                    ap_t[D:128, D:128], ee[:, 2 * hp + 1, :],
                    rr[:, 2 * hp + 1:2 * hp + 2])
                acc = psD.tile([128, 512], F32, name="acc_mp", tag="psD")
                last_pe = nc.tensor.matmul(
                    acc, ap_t, wp_sl(hp, slice(0, C)), start=True, stop=True)
                last_dve = nc.vector.tensor_copy(mpT[:, hp, :], acc)

            # M_pvT[c', c] = sum_(he) W_v[(he), c'] * M_pT[(he), c]
            for cp in range(KT):
                acc = psD.tile([128, 512], F32, name="acc_mpv", tag="psD")
                for kt in range(KT):
                    last_pe = nc.tensor.matmul(
                        acc,
                        wv_sl(kt, slice(cp * 128, (cp + 1) * 128)),
                        mpT[:, kt, :],
                        start=(kt == 0), stop=(kt == KT - 1),
                    )
                last_dve = nc.vector.tensor_copy(mpvT[:, cp, :], acc)

            # ---- P4: y = M_pv @ xf + bias ------------------------------
            # single-use half-strip tiles: no reuse => no WAR/WAW waits;
            # stores alternate between the SP and ACT HWDGE rings so each
            # ring stays within its 8 lanes
            for ym in range(KT):
                for half in range(2):
                    yh = const.tile([128, HW // 2], F16,
                                    name=f"yh{b}_{ym}_{half}")
                    for nbi in range(NB // 2):
                        nb = half * (NB // 2) + nbi
                        acc = psA.tile([128, 512], F32, name="acc_y",
                                       tag="psA")
                        for kt in range(KT):
                            last_pe = nc.tensor.matmul(
                                acc,
                                mpvT[:, kt, ym * 128:(ym + 1) * 128],
                                xf[:, kt, nb * 512:(nb + 1) * 512],
                                start=(kt == 0), stop=(kt == KT - 1),
                            )
                        last_act = nc.scalar.activation(
                            yh[:, nbi * 512:(nbi + 1) * 512], acc,
                            AF.Identity, bias=bias_ap(ym))
                    if ym == KT - 1 and half == 1:
                        eng = nc.gpsimd  # 9th HWDGE DMA would wrap a lane
                    elif half == 0:
                        eng = nc.sync
                    else:
                        eng = nc.scalar
                    tail.append(eng.dma_start(
                        out=ys[b][ym * 128:(ym + 1) * 128,
                                  half * (HW // 2):(half + 1) * (HW // 2)],
                        in_=yh))

        # ---- tail: SP observes every outstanding proc (1 wait per nop)
        for inst in [*tail, last_pe, last_act, last_dve]:
            if inst is None:
                continue
            n_ = nc.sync.nop(nofuse=True)
            tile.add_dep_helper(n_.ins, inst.ins, reason="tail observe")

    return nc


_NC_CACHE = None


def kernel(x, w_qkv, w_proj, b_proj):
    global _NC_CACHE
    if _NC_CACHE is None:
        _NC_CACHE = _build()
    nc = _NC_CACHE

    # one-pass fp32->fp16 cast (same rounding the on-device cast applied)
    x = np.asarray(x, dtype=np.float16).reshape(B, C, HW)
    w_qkv = np.asarray(w_qkv, dtype=np.float32)
    # interleave q_h / k_h row blocks so qkT columns are [q0|k0|q1|k1|...]
    perm = []
    for h in range(HEADS):
        perm.extend(range(h * D, (h + 1) * D))          # q_h rows
        perm.extend(range(C + h * D, C + (h + 1) * D))  # k_h rows
    w_qkT = w_qkv[perm].T                               # [512, 1024]
    w_v = w_qkv[2 * C:]                                 # [512, 512] natural
    w_pT = np.asarray(w_proj, dtype=np.float32).T
    b_col = np.asarray(b_proj, dtype=np.float32).reshape(C, 1)
    wall = np.ascontiguousarray(
        np.concatenate([w_qkT, w_v, w_pT, b_col], axis=1)).astype(
            np.float16)  # [512, 2049]; same rounding the on-device cast did

    outs = []
    for launch in range(2):
        in_maps = []
        for core in range(N_CORES):
            bi = launch * N_CORES + core
            in_maps.append({
                "x": np.ascontiguousarray(x[bi:bi + 1]),
                "wall": wall,
            })
        res = run_bass_kernel_spmd(nc, in_maps, core_ids=list(range(N_CORES)))
        outs.extend(r["y0"] for r in res.results)
    out = np.stack(outs)
    return out.reshape(B, C, 64, 64).astype(np.float32)
